# revision 1
# baseline (speedup 1.0000x reference)
"""CollisionRegularizer loss on 8 Trainium2 cores.

Strategy: every pairwise quantity (dist^2, the 6 scaled rotated-radius
projections, the velocity-approach dot) is a low-rank bilinear form in
per-point features, so they are computed as small-K matmuls on the PE
(host-prepped augmented feature rows), followed by a short elementwise
chain on DVE/ACT. Sharding: core c handles batch c//2, row-half c%2.
Each core emits per-partition partial sums; the host reduces.
"""

import numpy as np

import concourse.bacc as bacc
import concourse.mybir as mybir
from concourse import tile
from concourse.bass_utils import run_bass_kernel_spmd

B, N = 4, 2048
NC = 8
ROWS = 1024          # n-rows per core
NT = ROWS // 128     # 8 partition tiles
MC = 2               # m-chunks per row-tile
CHUNK = N // MC      # 1024 free-dim chain width
F32 = mybir.dt.float32

MM_TYPES = ["d2", "va", "su0", "su1", "su2", "sv0", "sv1", "sv2"]


def _quat_to_rotmat(q):
    qw, qx, qy, qz = q[..., 0], q[..., 1], q[..., 2], q[..., 3]
    R = np.stack(
        [
            1 - 2 * qy**2 - 2 * qz**2, 2 * qx * qy - 2 * qz * qw, 2 * qx * qz + 2 * qy * qw,
            2 * qx * qy + 2 * qz * qw, 1 - 2 * qx**2 - 2 * qz**2, 2 * qy * qz - 2 * qx * qw,
            2 * qx * qz - 2 * qy * qw, 2 * qy * qz + 2 * qx * qw, 1 - 2 * qx**2 - 2 * qy**2,
        ],
        axis=-1,
    )
    return R.reshape(*q.shape[:-1], 3, 3)


def _prep(xyz, scales, rotations, velocities):
    x = xyz.astype(np.float64)
    s = scales.astype(np.float64)
    v = velocities.astype(np.float64)
    R = _quat_to_rotmat(rotations.astype(np.float64))      # (B,N,3,3)
    a = np.einsum("bni,bnij->bnj", x, R)                   # x_n . R[n][:,j]
    c = (v * x).sum(-1)                                    # v_n . x_n
    nrm = (x * x).sum(-1)

    rhs = np.empty((B, 33, N), np.float32)
    rhs[:, 0:3] = x.transpose(0, 2, 1)
    rhs[:, 3] = 1.0
    rhs[:, 4] = nrm
    rhs[:, 5:8] = v.transpose(0, 2, 1)
    rhs[:, 8] = c
    for j in range(3):
        b0 = 9 + 4 * j
        rhs[:, b0:b0 + 3] = (x * s[:, :, j:j + 1]).transpose(0, 2, 1)
        rhs[:, b0 + 3] = s[:, :, j]
    for j in range(3):
        b0 = 21 + 4 * j
        rhs[:, b0:b0 + 3] = R[:, :, :, j].transpose(0, 2, 1)
        rhs[:, b0 + 3] = a[:, :, j]

    lhs = np.zeros((B, 8, 33, N), np.float32)
    lhs[:, 0, 0:3] = (-2.0 * x).transpose(0, 2, 1)
    lhs[:, 0, 3] = nrm + 1e-8
    lhs[:, 0, 4] = 1.0
    lhs[:, 1, 0:3] = v.transpose(0, 2, 1)
    lhs[:, 1, 3] = -c
    lhs[:, 1, 5:8] = x.transpose(0, 2, 1)
    lhs[:, 1, 8] = -1.0
    for j in range(3):
        b0 = 9 + 4 * j
        lhs[:, 2 + j, b0:b0 + 3] = R[:, :, :, j].transpose(0, 2, 1)
        lhs[:, 2 + j, b0 + 3] = -a[:, :, j]
    for j in range(3):
        b0 = 21 + 4 * j
        lhs[:, 5 + j, b0:b0 + 3] = (x * s[:, :, j:j + 1]).transpose(0, 2, 1)
        lhs[:, 5 + j, b0 + 3] = -s[:, :, j]
    return rhs, lhs


_NC_CACHE = {}

# perf config
F32R = True        # reduced-precision single-pass fp32 matmuls on PE
CHAIN_FP16 = True  # fp16 elementwise chain (2x/4x DVE throughput)
CLAMP = 1e-4       # dist^2 floor; keeps duplicates/diagonal harmless and
                   # bounds inv<=100 so every fp16 intermediate stays in range
F16 = mybir.dt.float16

# engine assignment for flexible elementwise ops: "dve" or "pool"
ASSIGN = {
    "add_r1s": "dve", "add_r2s": "dve", "rsum": "dve",
    "t": "dve", "ovp": "dve", "ov": "act", "den": "dve",
    "sqov": "dve", "g": "dve",
}


def _build(reps=1):
    key = (reps, F32R, CHAIN_FP16, tuple(sorted(ASSIGN.items())))
    if key in _NC_CACHE:
        return _NC_CACHE[key]
    CT = F16 if CHAIN_FP16 else F32
    MMT = mybir.dt.float32r if F32R else F32
    AF = mybir.ActivationFunctionType
    nc = bacc.Bacc(None, target_bir_lowering=False, debug=False)

    def _eng(k):
        return nc.gpsimd if ASSIGN[k] == "pool" else nc.vector

    rhs_d = nc.dram_tensor("rhs", [33, N], MMT, kind="ExternalInput")
    lhs_d = nc.dram_tensor("lhs", [8, 33, ROWS], MMT, kind="ExternalInput")
    rhs32_d = nc.dram_tensor("rhs32", [5, N], F32, kind="ExternalInput")
    lhs32_d = nc.dram_tensor("lhs32", [5, ROWS], F32, kind="ExternalInput")
    out_d = nc.dram_tensor("out", [128, 2 * NT * MC], F32, kind="ExternalOutput")

    with tile.TileContext(nc) as tc:
        with (
            tc.tile_pool(name="io", bufs=1) as io,
            tc.tile_pool(name="wk", bufs=3) as wk,
            tc.tile_pool(name="ch", bufs=3) as ch,
            tc.tile_pool(name="ps", bufs=4, space="PSUM") as ps,
        ):
            rhs_s = io.tile([33, N], MMT)
            nc.sync.dma_start(rhs_s[:], rhs_d[:])
            lhs_t = {}
            for ti, name in enumerate(MM_TYPES):
                lhs_t[name] = io.tile([33, ROWS], MMT, name="lhs_" + name)
                nc.sync.dma_start(lhs_t[name][:], lhs_d[ti])
            rhs32_s = io.tile([5, N], F32)
            nc.sync.dma_start(rhs32_s[:], rhs32_d[:])
            lhs32_s = io.tile([5, ROWS], F32)
            nc.sync.dma_start(lhs32_s[:], lhs32_d[:])
            ocols = io.tile([128, 2 * NT * MC], F32)

            from contextlib import nullcontext
            loop_cm = tc.For_i(0, reps, 1) if reps > 1 else nullcontext()
            with loop_cm:
              for nt in range(NT):
                nsl = slice(nt * 128, (nt + 1) * 128)
                for mc in range(MC):
                    it = nt * MC + mc
                    pt = {}
                    for name in MM_TYPES:
                        p = ps.tile([128, CHUNK], F32, name="p_" + name, tag="mm")
                        for h in range(CHUNK // 512):
                            m0 = mc * CHUNK + h * 512
                            if name == "d2":
                                nc.tensor.matmul(
                                    p[:, h * 512:(h + 1) * 512],
                                    lhs32_s[:, nsl],
                                    rhs32_s[:, m0:m0 + 512],
                                    start=True, stop=True,
                                )
                            else:
                                nc.tensor.matmul(
                                    p[:, h * 512:(h + 1) * 512],
                                    lhs_t[name][:, nsl],
                                    rhs_s[:, m0:m0 + 512],
                                    start=True, stop=True,
                                )
                        pt[name] = p

                    # PSUM drains
                    d2c = wk.tile([128, CHUNK], CT)
                    nc.vector.tensor_scalar_max(d2c[:], pt["d2"][:], CLAMP)
                    rva = wk.tile([128, CHUNK], CT)
                    nc.scalar.activation(rva[:], pt["va"][:], AF.Relu, scale=0.1)
                    # r1s via ACT squares (DVE cannot square PSUM) + Pool adds
                    squ = []
                    for j in range(3):
                        sq = wk.tile([128, CHUNK], CT, name=f"squ{j}")
                        nc.scalar.activation(sq[:], pt[f"su{j}"][:], AF.Square)
                        squ.append(sq)
                    r1s = wk.tile([128, CHUNK], CT)
                    _eng("add_r1s").tensor_add(r1s[:], squ[0][:], squ[1][:])
                    _eng("add_r1s").tensor_add(r1s[:], r1s[:], squ[2][:])
                    # r2s via ACT squares + Pool adds
                    sqv = []
                    for j in range(3):
                        sq = wk.tile([128, CHUNK], CT, name=f"sqv{j}")
                        nc.scalar.activation(sq[:], pt[f"sv{j}"][:], AF.Square)
                        sqv.append(sq)
                    r2s = wk.tile([128, CHUNK], CT)
                    _eng("add_r2s").tensor_add(r2s[:], sqv[0][:], sqv[1][:])
                    _eng("add_r2s").tensor_add(r2s[:], r2s[:], sqv[2][:])

                    dist = wk.tile([128, CHUNK], CT)
                    nc.scalar.activation(dist[:], d2c[:], AF.Sqrt)
                    inv = wk.tile([128, CHUNK], CT)
                    with nc.allow_low_precision("fp16 chain: inv<=100, rel err 5e-4"):
                        nc.vector.reciprocal(inv[:], dist[:])
                    r1 = ch.tile([128, CHUNK], CT)
                    nc.scalar.activation(r1[:], r1s[:], AF.Sqrt)
                    r2 = ch.tile([128, CHUNK], CT)
                    nc.scalar.activation(r2[:], r2s[:], AF.Sqrt)

                    rsum = ch.tile([128, CHUNK], CT)
                    _eng("rsum").tensor_add(rsum[:], r1[:], r2[:])
                    t = ch.tile([128, CHUNK], CT)
                    _eng("t").tensor_mul(t[:], rsum[:], inv[:])
                    ovp = ch.tile([128, CHUNK], CT)
                    _eng("ovp").tensor_sub(ovp[:], t[:], dist[:])
                    ov = wk.tile([128, CHUNK], CT)
                    if ASSIGN["ov"] == "act":
                        nc.scalar.activation(ov[:], ovp[:], AF.Relu)
                    else:
                        _eng("ov").tensor_scalar_max(ov[:], ovp[:], 0.0)

                    den = ch.tile([128, CHUNK], CT)
                    if ASSIGN["den"] == "act":
                        nc.scalar.activation(den[:], ov[:], AF.Identity,
                                             bias=1.0, scale=0.1)
                    else:
                        _eng("den").tensor_scalar(den[:], ov[:], 0.1, 1.0,
                                                  mybir.AluOpType.mult,
                                                  mybir.AluOpType.add)
                    rden = ch.tile([128, CHUNK], CT)
                    with nc.allow_low_precision("fp16 chain"):
                        nc.vector.reciprocal(rden[:], den[:])
                    sqov = ch.tile([128, CHUNK], CT)
                    if ASSIGN["sqov"] == "act":
                        nc.scalar.activation(sqov[:], ov[:], AF.Square)
                    else:
                        _eng("sqov").tensor_mul(sqov[:], ov[:], ov[:])
                    spec = ch.tile([128, CHUNK], CT)
                    nc.vector.scalar_tensor_tensor(
                        out=spec[:], in0=sqov[:], scalar=1.0, in1=rden[:],
                        op0=mybir.AluOpType.mult, op1=mybir.AluOpType.mult,
                        accum_out=ocols[:, 2 * it:2 * it + 1])

                    g = ch.tile([128, CHUNK], CT)
                    _eng("g").tensor_mul(g[:], ov[:], inv[:])
                    vt = ch.tile([128, CHUNK], CT)
                    nc.vector.scalar_tensor_tensor(
                        out=vt[:], in0=g[:], scalar=1.0, in1=rva[:],
                        op0=mybir.AluOpType.mult, op1=mybir.AluOpType.mult,
                        accum_out=ocols[:, 2 * it + 1:2 * it + 2])

            nc.sync.dma_start(out_d[:], ocols[:])

    nc.compile()
    _NC_CACHE[key] = nc
    return nc


def make_in_maps(xyz, scales, rotations, velocities):
    rhs, lhs = _prep(xyz, scales, rotations, velocities)
    in_maps = []
    for c in range(NC):
        b, half = c // 2, c % 2
        in_maps.append({
            "rhs": np.ascontiguousarray(rhs[b]),
            "lhs": np.ascontiguousarray(lhs[b][:, :, half * ROWS:(half + 1) * ROWS]),
            "rhs32": np.ascontiguousarray(rhs[b][0:5]),
            "lhs32": np.ascontiguousarray(lhs[b][0, 0:5, half * ROWS:(half + 1) * ROWS]),
        })
    return in_maps


def finish(results):
    total = 0.0
    for c in range(NC):
        total += results[c]["out"].astype(np.float64).sum()
    return np.float32(total / (B * N * N))


_RUNNER = {}


def _get_runner(reps=1):
    """Cached shard_map-jitted executor (mirrors bass2jax.run_bass_via_pjrt
    multi-core path) so repeated calls skip re-compilation."""
    if reps in _RUNNER:
        return _RUNNER[reps]
    import jax
    from jax.sharding import Mesh, PartitionSpec
    from jax.experimental.shard_map import shard_map
    from concourse import bass2jax

    nc = _build(reps)
    bass2jax.install_neuronx_cc_hook()

    part_name = nc.partition_id_tensor.name if nc.partition_id_tensor else None
    in_names, out_names, out_avals, zero_outs = [], [], [], []
    for alloc in nc.m.functions[0].allocations:
        if not isinstance(alloc, mybir.MemoryLocationSet):
            continue
        name = alloc.memorylocations[0].name
        if alloc.kind == "ExternalInput":
            if name != part_name:
                in_names.append(name)
        elif alloc.kind == "ExternalOutput":
            out_names.append(name)
            shape = tuple(alloc.tensor_shape)
            dtype = mybir.dt.np(alloc.dtype)
            out_avals.append(jax.core.ShapedArray(shape, dtype))
            zero_outs.append(np.zeros(shape, dtype))
    n_params = len(in_names)
    all_names = in_names + out_names
    if part_name is not None:
        all_names = all_names + [part_name]

    def _body(*args):
        operands = list(args)
        if part_name is not None:
            operands.append(bass2jax.partition_id_tensor())
        outs = bass2jax._bass_exec_p.bind(
            *operands,
            out_avals=tuple(out_avals),
            in_names=tuple(all_names),
            out_names=tuple(out_names),
            lowering_input_output_aliases=(),
            sim_require_finite=True,
            sim_require_nnan=True,
            nc=nc,
        )
        return tuple(outs)

    devices = jax.devices()[:NC]
    mesh = Mesh(np.asarray(devices), ("core",))
    n_outs = len(out_names)
    fn = jax.jit(
        shard_map(
            _body, mesh=mesh,
            in_specs=(PartitionSpec("core"),) * (n_params + n_outs),
            out_specs=(PartitionSpec("core"),) * n_outs,
            check_rep=False,
        ),
        donate_argnums=tuple(range(n_params, n_params + n_outs)),
        keep_unused=True,
    )

    def run(in_maps):
        concat_in = [
            np.concatenate([in_maps[c][nm] for c in range(NC)], axis=0)
            for nm in in_names
        ]
        concat_zeros = [
            np.zeros((NC * z.shape[0], *z.shape[1:]), z.dtype) for z in zero_outs
        ]
        out_arrs = fn(*concat_in, *concat_zeros)
        return [
            {nm: np.asarray(out_arrs[i]).reshape(NC, *out_avals[i].shape)[c]
             for i, nm in enumerate(out_names)}
            for c in range(NC)
        ]

    _RUNNER[reps] = run
    return run


def kernel(xyz, scales, rotations, velocities):
    run = _get_runner()
    in_maps = make_in_maps(xyz, scales, rotations, velocities)
    return finish(run(in_maps))


if __name__ == "__main__":
    rng = np.random.default_rng(0)
    ins = {
        "xyz": rng.standard_normal((B, N, 3)).astype(np.float32),
        "scales": rng.random((B, N, 3)).astype(np.float32),
        "rotations": rng.standard_normal((B, N, 4)).astype(np.float32),
        "velocities": rng.standard_normal((B, N, 3)).astype(np.float32),
    }
    print(kernel(**ins))



# revision 8
# speedup vs baseline: 1.7237x; 1.7237x over previous
"""CollisionRegularizer loss on 8 Trainium2 cores.

v2: exploits the n<->m symmetry of the pairwise integrand (each unordered
pair computed once: row-tile r covers column-blocks r..r+8 mod 16, strict
upper mask on the diagonal block, antipodal block only for r<8), and
computes r1s/r2s directly as K=30 quadratic forms on the PE (replacing
6 ACT squares + 4 DVE adds per tile). The spectral term is accumulated
via sum(u - 2 + 1/u) = 0.01*sum(ov^2/(1+0.1 ov)), u = 1+0.1*ov, so only
three running sums (den, rden, vta) leave the chip. fp16 chain; NaN from
sqrt(fp32r-noise<0) is suppressed by max(NaN,0)=0 on DVE (verified).
Sharding: core c handles batch c//2; row-tile set A/B by c%2.
"""

import numpy as np

import concourse.bacc as bacc
import concourse.mybir as mybir
from concourse import tile
from concourse.bass_utils import run_bass_kernel_spmd

B, N = 4, 2048
NC = 8
EPS = 1e-5          # dist^2 floor folded into the d2 bilinear form
F32 = mybir.dt.float32
F32R = mybir.dt.float32r
F16 = mybir.dt.float16

ROWSETS = ([0, 1, 2, 3, 12, 13, 14, 15], [4, 5, 6, 7, 8, 9, 10, 11])
# fixed per-strip rhs base offsets: strip i reads rhs columns [OFF, OFF+W).
# Host rolls the rhs feature matrix per core so strips never wrap.
STRIP_OFF = [0, 1280, 2560, 3840, 5120, 6272, 7424, 8576]
RHS_COLS = 9600
SYM = [(0, 0), (1, 1), (2, 2), (0, 1), (0, 2), (1, 2)]

# engine for flexible ops: "dve" | "pool" | "act" (drains)
# Pool can only run plain TensorTensor ops on HW (TSP/STT fail the
# walrus ISA check); all accumulating ops live on DVE/ACT.
ASSIGN = {
    "s1": "pool", "t": "dve", "ovp": "dve", "h": "pool",
    "rva": "act", "WKBUFS": 2,
}


def _quat_to_rotmat(q):
    qw, qx, qy, qz = q[..., 0], q[..., 1], q[..., 2], q[..., 3]
    R = np.stack(
        [
            1 - 2 * qy**2 - 2 * qz**2, 2 * qx * qy - 2 * qz * qw, 2 * qx * qz + 2 * qy * qw,
            2 * qx * qy + 2 * qz * qw, 1 - 2 * qx**2 - 2 * qz**2, 2 * qy * qz - 2 * qx * qw,
            2 * qx * qz - 2 * qy * qw, 2 * qy * qz + 2 * qx * qw, 1 - 2 * qx**2 - 2 * qy**2,
        ],
        axis=-1,
    )
    return R.reshape(*q.shape[:-1], 3, 3)


def _prep(xyz, scales, rotations, velocities):
    x = xyz.astype(np.float64)
    s = scales.astype(np.float64)
    v = velocities.astype(np.float64)
    R = _quat_to_rotmat(rotations.astype(np.float64))
    a = np.einsum("bni,bnij->bnj", x, R)
    c = (v * x).sum(-1)
    nrm = (x * x).sum(-1)
    s2 = s * s

    l32 = np.empty((B, 5, N))
    r32 = np.empty((B, 5, N))
    l32[:, 0] = nrm + EPS
    l32[:, 1] = 1.0
    l32[:, 2:5] = (-2.0 * x).transpose(0, 2, 1)
    r32[:, 0] = 1.0
    r32[:, 1] = nrm
    r32[:, 2:5] = x.transpose(0, 2, 1)

    lr = np.zeros((B, 96, N))
    rr = np.zeros((B, 96, N))
    # va rows 0:8 (base partition 0)
    lr[:, 0] = -c
    lr[:, 1:4] = v.transpose(0, 2, 1)
    lr[:, 4:7] = x.transpose(0, 2, 1)
    lr[:, 7] = 1.0
    rr[:, 0] = 1.0
    rr[:, 1:4] = x.transpose(0, 2, 1)
    rr[:, 4:7] = v.transpose(0, 2, 1)
    rr[:, 7] = -c
    # r1s rows 32:62 (base 32), r2s rows 64:94 (base 64)
    for j in range(3):
        o = 32 + 10 * j
        Rj = R[:, :, :, j]
        lr[:, o] = a[:, :, j] ** 2
        lr[:, o + 1:o + 4] = (a[:, :, j:j + 1] * Rj).transpose(0, 2, 1)
        rr[:, o] = s2[:, :, j]
        rr[:, o + 1:o + 4] = (-2.0 * s2[:, :, j:j + 1] * x).transpose(0, 2, 1)
        o2 = 64 + 10 * j
        rr[:, o2] = a[:, :, j] ** 2
        rr[:, o2 + 1:o2 + 4] = (a[:, :, j:j + 1] * Rj).transpose(0, 2, 1)
        lr[:, o2] = s2[:, :, j]
        lr[:, o2 + 1:o2 + 4] = (-2.0 * s2[:, :, j:j + 1] * x).transpose(0, 2, 1)
        for k, (p, q) in enumerate(SYM):
            dbl = 1.0 if p == q else 2.0
            lr[:, o + 4 + k] = Rj[:, :, p] * Rj[:, :, q]
            rr[:, o + 4 + k] = dbl * s2[:, :, j] * x[:, :, p] * x[:, :, q]
            rr[:, o2 + 4 + k] = Rj[:, :, p] * Rj[:, :, q]
            lr[:, o2 + 4 + k] = dbl * s2[:, :, j] * x[:, :, p] * x[:, :, q]
    return l32, r32, lr, rr


def _strip_plan(half):
    """[(strip_idx, rowtile, col_start, width, chunks=[(u0,cw),...])]"""
    plan = []
    for i, rt in enumerate(ROWSETS[half]):
        W = 1152 if rt < 8 else 1024
        chunks = [(0, 512), (512, 512)] + ([(1024, 128)] if W == 1152 else [])
        plan.append((i, rt, (128 * rt) % N, W, chunks))
    return plan


_NC_CACHE = {}


def _build(reps=1):
    key = (reps, tuple(sorted(ASSIGN.items())))
    if key in _NC_CACHE:
        return _NC_CACHE[key]
    AF = mybir.ActivationFunctionType
    ALU = mybir.AluOpType
    nc = bacc.Bacc(None, target_bir_lowering=False, debug=False)

    l32_d = nc.dram_tensor("l32", [5, 1024], F32, kind="ExternalInput")
    r32_d = nc.dram_tensor("r32", [5, RHS_COLS], F32, kind="ExternalInput")
    lr_d = nc.dram_tensor("lr", [96, 1024], F32R, kind="ExternalInput")
    rr_d = nc.dram_tensor("rr", [96, RHS_COLS], F32R, kind="ExternalInput")
    msk_d = nc.dram_tensor("msk", [128, 128], F16, kind="ExternalInput")
    out_d = nc.dram_tensor("out", [128, 24], F32, kind="ExternalOutput")

    def eng(k):
        return {"dve": nc.vector, "pool": nc.gpsimd, "act": nc.scalar}[ASSIGN[k]]

    with tile.TileContext(nc) as tc:
        with (
            tc.tile_pool(name="io", bufs=1) as io,
            tc.tile_pool(name="wk", bufs=ASSIGN["WKBUFS"]) as wk,
            tc.tile_pool(name="ps", bufs=8, space="PSUM") as ps,
        ):
            l32_s = io.tile([5, 1024], F32)
            nc.sync.dma_start(l32_s[:], l32_d[:])
            r32_s = io.tile([5, RHS_COLS], F32)
            nc.sync.dma_start(r32_s[:], r32_d[:])
            lr_s = io.tile([96, 1024], F32R)
            nc.sync.dma_start(lr_s[:], lr_d[:])
            rr_s = io.tile([96, RHS_COLS], F32R)
            nc.sync.dma_start(rr_s[:], rr_d[:])
            msk_s = io.tile([128, 128], F16)
            nc.sync.dma_start(msk_s[:], msk_d[:])
            ones = io.tile([128, 1152], F16)
            nc.vector.memset(ones[:], 1.0)
            ocols = io.tile([128, 24], F32)

            # both halves share the same strip-shape plan (widths per
            # strip index are equal); per-core column starts are baked
            # into the host-rolled rhs layout at STRIP_OFF.
            plan = _strip_plan(0)

            from contextlib import nullcontext
            loop_cm = tc.For_i(0, reps, 1) if reps > 1 else nullcontext()
            with loop_cm:
                for i, rt, s0, W, chunks in plan:
                    nsl = slice(128 * i, 128 * (i + 1))
                    # host pre-rolls rhs so this strip's columns start at
                    # fixed offset STRIP_OFF[i] (see make_in_maps)
                    base = STRIP_OFF[i]
                    dist = wk.tile([128, 1152], F16, name="dist")
                    r1 = wk.tile([128, 1152], F16, name="r1")
                    r2 = wk.tile([128, 1152], F16, name="r2")
                    rva = wk.tile([128, 1152], F16, name="rva")
                    for u0, cw in chunks:
                        g0 = base + u0
                        p_d2 = ps.tile([128, 512], F32, name="p_d2", tag="mm")
                        p_va = ps.tile([128, 512], F32, name="p_va", tag="mm")
                        p_r1 = ps.tile([128, 512], F32, name="p_r1", tag="mm")
                        p_r2 = ps.tile([128, 512], F32, name="p_r2", tag="mm")
                        nc.tensor.matmul(p_d2[:, :cw], l32_s[:, nsl],
                                         r32_s[:, g0:g0 + cw],
                                         start=True, stop=True)
                        nc.tensor.matmul(p_va[:, :cw], lr_s[0:8, nsl],
                                         rr_s[0:8, g0:g0 + cw],
                                         start=True, stop=True)
                        nc.tensor.matmul(p_r1[:, :cw], lr_s[32:62, nsl],
                                         rr_s[32:62, g0:g0 + cw],
                                         start=True, stop=True)
                        nc.tensor.matmul(p_r2[:, :cw], lr_s[64:94, nsl],
                                         rr_s[64:94, g0:g0 + cw],
                                         start=True, stop=True)
                        csl = slice(u0, u0 + cw)
                        nc.scalar.activation(dist[:, csl], p_d2[:, :cw], AF.Sqrt)
                        nc.scalar.activation(r1[:, csl], p_r1[:, :cw], AF.Sqrt)
                        nc.scalar.activation(r2[:, csl], p_r2[:, :cw], AF.Sqrt)
                        if ASSIGN["rva"] == "act":
                            nc.scalar.activation(rva[:, csl], p_va[:, :cw],
                                                 AF.Relu, scale=0.1)
                        else:
                            nc.gpsimd.tensor_scalar(rva[:, csl], p_va[:, :cw],
                                                    0.1, 0.0, ALU.mult, ALU.max)

                    w = slice(0, W)
                    inv = wk.tile([128, 1152], F16, name="inv")
                    with nc.allow_low_precision("fp16 chain, inv<=320"):
                        nc.vector.reciprocal(inv[:, w], dist[:, w])
                    s1 = wk.tile([128, 1152], F16, name="s1")
                    eng("s1").tensor_add(s1[:, w], r1[:, w], r2[:, w])
                    t = wk.tile([128, 1152], F16, name="t")
                    eng("t").tensor_mul(t[:, w], s1[:, w], inv[:, w])
                    ovp = wk.tile([128, 1152], F16, name="ovp")
                    eng("ovp").tensor_sub(ovp[:, w], t[:, w], dist[:, w])
                    # mask before max: NaN*0=NaN then max(NaN,0)=0
                    nc.vector.tensor_mul(ovp[:, 0:128], ovp[:, 0:128], msk_s[:])
                    ov = wk.tile([128, 1152], F16, name="ov")
                    with nc.allow_low_precision("fp16 chain"):
                        nc.vector.tensor_scalar_max(ov[:, w], ovp[:, w], 0.0)
                        den = wk.tile([128, 1152], F16, name="den")
                        nc.vector.scalar_tensor_tensor(
                            out=den[:, w], in0=ov[:, w], scalar=0.1,
                            in1=ones[:, w], op0=ALU.mult, op1=ALU.add,
                            accum_out=ocols[:, 3 * i:3 * i + 1])
                        rden = wk.tile([128, 1152], F16, name="rden")
                        nc.vector.reciprocal(rden[:, w], den[:, w])
                        rd2 = wk.tile([128, 1152], F16, name="rd2")
                        nc.scalar.activation(
                            rd2[:, w], rden[:, w], AF.Relu,
                            accum_out=ocols[:, 3 * i + 1:3 * i + 2])
                    h = wk.tile([128, 1152], F16, name="h")
                    eng("h").tensor_mul(h[:, w], ov[:, w], inv[:, w])
                    vta = wk.tile([128, 1152], F16, name="vta")
                    nc.vector.scalar_tensor_tensor(
                        out=vta[:, w], in0=h[:, w], scalar=1.0, in1=rva[:, w],
                        op0=ALU.mult, op1=ALU.mult,
                        accum_out=ocols[:, 3 * i + 2:3 * i + 3])

            nc.sync.dma_start(out_d[:], ocols[:])

    nc.compile()
    _NC_CACHE[key] = nc
    return nc


def make_in_maps(xyz, scales, rotations, velocities):
    l32, r32, lr, rr = _prep(xyz, scales, rotations, velocities)
    mask = (np.arange(128)[None, :] > np.arange(128)[:, None])
    mask = mask.astype(np.float16)
    in_maps = []
    for c in range(NC):
        b, half = c // 2, c % 2
        rts = ROWSETS[half]
        lcols = np.concatenate([np.arange(128 * rt, 128 * (rt + 1))
                                for rt in rts])
        r32c = np.empty((5, RHS_COLS), np.float32)
        rrc = np.zeros((96, RHS_COLS), np.float32)
        for i, rt in enumerate(rts):
            W = 1152 if rt < 8 else 1024
            cols = (128 * rt + np.arange(W)) % N
            o = STRIP_OFF[i]
            r32c[:, o:o + W] = r32[b][:, cols]
            rrc[:, o:o + W] = rr[b][:, cols]
        in_maps.append({
            "l32": np.ascontiguousarray(l32[b][:, lcols]).astype(np.float32),
            "r32": r32c,
            "lr": np.ascontiguousarray(lr[b][:, lcols]).astype(np.float32),
            "rr": rrc,
            "msk": mask,
        })
    return in_maps


def finish(results):
    total = 0.0
    C_core = 4 * 128 * 1152 + 4 * 128 * 1024
    for c in range(NC):
        o = results[c]["out"].astype(np.float64)
        sden = o[:, 0::3].sum()
        srden = o[:, 1::3].sum()
        svta = o[:, 2::3].sum()
        total += 100.0 * (sden + srden - 2.0 * C_core) + svta
    return np.float32(2.0 * total / (B * N * N))


_RUNNER = {}


def _get_runner(reps=1):
    """Cached shard_map-jitted executor (mirrors bass2jax.run_bass_via_pjrt
    multi-core path) so repeated calls skip re-compilation."""
    if reps in _RUNNER:
        return _RUNNER[reps]
    import jax
    from jax.sharding import Mesh, PartitionSpec
    from jax.experimental.shard_map import shard_map
    from concourse import bass2jax

    nc = _build(reps)
    bass2jax.install_neuronx_cc_hook()

    part_name = nc.partition_id_tensor.name if nc.partition_id_tensor else None
    in_names, out_names, out_avals, zero_outs = [], [], [], []
    for alloc in nc.m.functions[0].allocations:
        if not isinstance(alloc, mybir.MemoryLocationSet):
            continue
        name = alloc.memorylocations[0].name
        if alloc.kind == "ExternalInput":
            if name != part_name:
                in_names.append(name)
        elif alloc.kind == "ExternalOutput":
            out_names.append(name)
            shape = tuple(alloc.tensor_shape)
            dtype = mybir.dt.np(alloc.dtype)
            out_avals.append(jax.core.ShapedArray(shape, dtype))
            zero_outs.append(np.zeros(shape, dtype))
    n_params = len(in_names)
    all_names = in_names + out_names
    if part_name is not None:
        all_names = all_names + [part_name]

    def _body(*args):
        operands = list(args)
        if part_name is not None:
            operands.append(bass2jax.partition_id_tensor())
        outs = bass2jax._bass_exec_p.bind(
            *operands,
            out_avals=tuple(out_avals),
            in_names=tuple(all_names),
            out_names=tuple(out_names),
            lowering_input_output_aliases=(),
            sim_require_finite=True,
            sim_require_nnan=True,
            nc=nc,
        )
        return tuple(outs)

    devices = jax.devices()[:NC]
    mesh = Mesh(np.asarray(devices), ("core",))
    n_outs = len(out_names)
    fn = jax.jit(
        shard_map(
            _body, mesh=mesh,
            in_specs=(PartitionSpec("core"),) * (n_params + n_outs),
            out_specs=(PartitionSpec("core"),) * n_outs,
            check_rep=False,
        ),
        donate_argnums=tuple(range(n_params, n_params + n_outs)),
        keep_unused=True,
    )

    def run(in_maps):
        concat_in = [
            np.concatenate([in_maps[c][nm] for c in range(NC)], axis=0)
            for nm in in_names
        ]
        concat_zeros = [
            np.zeros((NC * z.shape[0], *z.shape[1:]), z.dtype) for z in zero_outs
        ]
        out_arrs = fn(*concat_in, *concat_zeros)
        return [
            {nm: np.asarray(out_arrs[i]).reshape(NC, *out_avals[i].shape)[c]
             for i, nm in enumerate(out_names)}
            for c in range(NC)
        ]

    _RUNNER[reps] = run
    return run


def kernel(xyz, scales, rotations, velocities):
    run = _get_runner()
    in_maps = make_in_maps(xyz, scales, rotations, velocities)
    return finish(run(in_maps))


if __name__ == "__main__":
    rng = np.random.default_rng(0)
    ins = {
        "xyz": rng.standard_normal((B, N, 3)).astype(np.float32),
        "scales": rng.random((B, N, 3)).astype(np.float32),
        "rotations": rng.standard_normal((B, N, 4)).astype(np.float32),
        "velocities": rng.standard_normal((B, N, 3)).astype(np.float32),
    }
    print(kernel(**ins))


# revision 11
# speedup vs baseline: 2.3754x; 1.3781x over previous
"""CollisionRegularizer loss on 8 Trainium2 cores.

v2: exploits the n<->m symmetry of the pairwise integrand (each unordered
pair computed once: row-tile r covers column-blocks r..r+8 mod 16, strict
upper mask on the diagonal block, antipodal block only for r<8), and
computes r1s/r2s directly as K=30 quadratic forms on the PE (replacing
6 ACT squares + 4 DVE adds per tile). The spectral term is accumulated
via sum(u - 2 + 1/u) = 0.01*sum(ov^2/(1+0.1 ov)), u = 1+0.1*ov, so only
three running sums (den, rden, vta) leave the chip. fp16 chain; NaN from
sqrt(fp32r-noise<0) is suppressed by max(NaN,0)=0 on DVE (verified).
Sharding: core c handles batch c//2; row-tile set A/B by c%2.
"""

import numpy as np

import concourse.bacc as bacc
import concourse.mybir as mybir
from concourse import tile
from concourse.bass_utils import run_bass_kernel_spmd

B, N = 4, 2048
NC = 8
EPS = 1e-5          # dist^2 floor folded into the d2 bilinear form
F32 = mybir.dt.float32
F32R = mybir.dt.float32r
F16 = mybir.dt.float16

ROWSETS = ([0, 1, 2, 3, 12, 13, 14, 15], [4, 5, 6, 7, 8, 9, 10, 11])
# fixed per-strip rhs base offsets: strip i reads rhs columns [OFF, OFF+W).
# Host rolls the rhs feature matrix per core so strips never wrap.
STRIP_OFF = [0, 1280, 2560, 3840, 5120, 6272, 7424, 8576]
RHS_COLS = 9600
SYM = [(0, 0), (1, 1), (2, 2), (0, 1), (0, 2), (1, 2)]

# engine for flexible ops: "dve" | "pool" | "act" (drains)
# Pool can only run plain TensorTensor ops on HW (TSP/STT fail the
# walrus ISA check); all accumulating ops live on DVE/ACT.
ASSIGN = {
    "s1": "pool", "t": "dve", "ovp": "dve", "h": "pool",
    "rva": "act", "WKBUFS": 2,
}


def _quat_to_rotmat(q):
    qw, qx, qy, qz = q[..., 0], q[..., 1], q[..., 2], q[..., 3]
    R = np.stack(
        [
            1 - 2 * qy**2 - 2 * qz**2, 2 * qx * qy - 2 * qz * qw, 2 * qx * qz + 2 * qy * qw,
            2 * qx * qy + 2 * qz * qw, 1 - 2 * qx**2 - 2 * qz**2, 2 * qy * qz - 2 * qx * qw,
            2 * qx * qz - 2 * qy * qw, 2 * qy * qz + 2 * qx * qw, 1 - 2 * qx**2 - 2 * qy**2,
        ],
        axis=-1,
    )
    return R.reshape(*q.shape[:-1], 3, 3)


def _prep(xyz, scales, rotations, velocities):
    x = xyz.astype(np.float64)
    s = scales.astype(np.float64)
    v = velocities.astype(np.float64)
    R = _quat_to_rotmat(rotations.astype(np.float64))
    a = np.einsum("bni,bnij->bnj", x, R)
    c = (v * x).sum(-1)
    nrm = (x * x).sum(-1)
    s2 = s * s

    l32 = np.empty((B, 5, N))
    r32 = np.empty((B, 5, N))
    l32[:, 0] = nrm + EPS
    l32[:, 1] = 1.0
    l32[:, 2:5] = (-2.0 * x).transpose(0, 2, 1)
    r32[:, 0] = 1.0
    r32[:, 1] = nrm
    r32[:, 2:5] = x.transpose(0, 2, 1)

    lr = np.zeros((B, 96, N))
    rr = np.zeros((B, 96, N))
    # va rows 0:8 (base partition 0)
    lr[:, 0] = -c
    lr[:, 1:4] = v.transpose(0, 2, 1)
    lr[:, 4:7] = x.transpose(0, 2, 1)
    lr[:, 7] = 1.0
    rr[:, 0] = 1.0
    rr[:, 1:4] = x.transpose(0, 2, 1)
    rr[:, 4:7] = v.transpose(0, 2, 1)
    rr[:, 7] = -c
    # r1s rows 32:62 (base 32), r2s rows 64:94 (base 64)
    for j in range(3):
        o = 32 + 10 * j
        Rj = R[:, :, :, j]
        lr[:, o] = a[:, :, j] ** 2
        lr[:, o + 1:o + 4] = (a[:, :, j:j + 1] * Rj).transpose(0, 2, 1)
        rr[:, o] = s2[:, :, j]
        rr[:, o + 1:o + 4] = (-2.0 * s2[:, :, j:j + 1] * x).transpose(0, 2, 1)
        o2 = 64 + 10 * j
        rr[:, o2] = a[:, :, j] ** 2
        rr[:, o2 + 1:o2 + 4] = (a[:, :, j:j + 1] * Rj).transpose(0, 2, 1)
        lr[:, o2] = s2[:, :, j]
        lr[:, o2 + 1:o2 + 4] = (-2.0 * s2[:, :, j:j + 1] * x).transpose(0, 2, 1)
        for k, (p, q) in enumerate(SYM):
            dbl = 1.0 if p == q else 2.0
            lr[:, o + 4 + k] = Rj[:, :, p] * Rj[:, :, q]
            rr[:, o + 4 + k] = dbl * s2[:, :, j] * x[:, :, p] * x[:, :, q]
            rr[:, o2 + 4 + k] = Rj[:, :, p] * Rj[:, :, q]
            lr[:, o2 + 4 + k] = dbl * s2[:, :, j] * x[:, :, p] * x[:, :, q]
    return l32, r32, lr, rr


def _strip_plan(half):
    """[(strip_idx, rowtile, col_start, width, chunks=[(u0,cw),...])]"""
    plan = []
    for i, rt in enumerate(ROWSETS[half]):
        W = 1152 if rt < 8 else 1024
        chunks = [(0, 512), (512, 512)] + ([(1024, 128)] if W == 1152 else [])
        plan.append((i, rt, (128 * rt) % N, W, chunks))
    return plan


_NC_CACHE = {}


def _build(reps=1):
    key = (reps, tuple(sorted(ASSIGN.items())))
    if key in _NC_CACHE:
        return _NC_CACHE[key]
    AF = mybir.ActivationFunctionType
    ALU = mybir.AluOpType
    nc = bacc.Bacc(None, target_bir_lowering=False, debug=False)

    l32_d = nc.dram_tensor("l32", [5, 1024], F32, kind="ExternalInput")
    r32_d = nc.dram_tensor("r32", [5, RHS_COLS], F32, kind="ExternalInput")
    lr_d = nc.dram_tensor("lr", [96, 1024], F32R, kind="ExternalInput")
    rr_d = nc.dram_tensor("rr", [96, RHS_COLS], F32R, kind="ExternalInput")
    msk_d = nc.dram_tensor("msk", [128, 128], F16, kind="ExternalInput")
    out_d = nc.dram_tensor("out", [128, 24], F32, kind="ExternalOutput")

    def eng(k):
        return {"dve": nc.vector, "pool": nc.gpsimd, "act": nc.scalar}[ASSIGN[k]]

    with tile.TileContext(nc) as tc:
        with (
            tc.tile_pool(name="io", bufs=1) as io,
            tc.tile_pool(name="wk", bufs=ASSIGN["WKBUFS"]) as wk,
            tc.tile_pool(name="ps", bufs=2, space="PSUM") as ps,
            tc.tile_pool(name="psb", bufs=2, space="PSUM") as psb,
        ):
            l32_s = io.tile([5, 1024], F32)
            nc.sync.dma_start(l32_s[:], l32_d[:])
            r32_s = io.tile([5, RHS_COLS], F32)
            nc.sync.dma_start(r32_s[:], r32_d[:])
            lr_s = io.tile([96, 1024], F32R)
            nc.sync.dma_start(lr_s[:], lr_d[:])
            rr_s = io.tile([96, RHS_COLS], F32R)
            nc.sync.dma_start(rr_s[:], rr_d[:])
            msk_s = io.tile([128, 128], F16)
            nc.sync.dma_start(msk_s[:], msk_d[:])
            ones = io.tile([128, 1152], F16)
            nc.vector.memset(ones[:], 1.0)
            ocols = io.tile([128, 24], F32)

            # both halves share the same strip-shape plan (widths per
            # strip index are equal); per-core column starts are baked
            # into the host-rolled rhs layout at STRIP_OFF.
            plan = _strip_plan(0)

            from contextlib import nullcontext
            loop_cm = tc.For_i(0, reps, 1) if reps > 1 else nullcontext()
            with loop_cm:
                for i, rt, s0, W, chunks in plan:
                    nsl = slice(128 * i, 128 * (i + 1))
                    # host pre-rolls rhs so this strip's columns start at
                    # fixed offset STRIP_OFF[i] (see make_in_maps)
                    base = STRIP_OFF[i]
                    dist = wk.tile([128, 1152], F16, name="dist")
                    r1 = wk.tile([128, 1152], F16, name="r1")
                    r2 = wk.tile([128, 1152], F16, name="r2")
                    rva = wk.tile([128, 1152], F16, name="rva")
                    p_r1 = psb.tile([128, 1152], F32, name="p_r1", tag="mmb")
                    p_r2 = psb.tile([128, 1152], F32, name="p_r2", tag="mmb")
                    for u0, cw in chunks:
                        g0 = base + u0
                        p_d2 = ps.tile([128, 512], F32, name="p_d2", tag="mm")
                        p_va = ps.tile([128, 512], F32, name="p_va", tag="mm")
                        nc.tensor.matmul(p_d2[:, :cw], l32_s[:, nsl],
                                         r32_s[:, g0:g0 + cw],
                                         start=True, stop=True)
                        nc.tensor.matmul(p_va[:, :cw], lr_s[0:8, nsl],
                                         rr_s[0:8, g0:g0 + cw],
                                         start=True, stop=True)
                        nc.tensor.matmul(p_r1[:, u0:u0 + cw], lr_s[32:62, nsl],
                                         rr_s[32:62, g0:g0 + cw],
                                         start=True, stop=True)
                        nc.tensor.matmul(p_r2[:, u0:u0 + cw], lr_s[64:94, nsl],
                                         rr_s[64:94, g0:g0 + cw],
                                         start=True, stop=True)
                        csl = slice(u0, u0 + cw)
                        nc.scalar.activation(dist[:, csl], p_d2[:, :cw], AF.Sqrt)
                        if ASSIGN["rva"] == "act":
                            nc.scalar.activation(rva[:, csl], p_va[:, :cw],
                                                 AF.Relu, scale=0.1)
                        else:
                            nc.gpsimd.tensor_scalar(rva[:, csl], p_va[:, :cw],
                                                    0.1, 0.0, ALU.mult, ALU.max)

                    w = slice(0, W)
                    nc.scalar.activation(r1[:, w], p_r1[:, w], AF.Sqrt)
                    nc.scalar.activation(r2[:, w], p_r2[:, w], AF.Sqrt)
                    inv = wk.tile([128, 1152], F16, name="inv")
                    with nc.allow_low_precision("fp16 chain, inv<=320"):
                        nc.vector.reciprocal(inv[:, w], dist[:, w])
                    s1 = wk.tile([128, 1152], F16, name="s1")
                    eng("s1").tensor_add(s1[:, w], r1[:, w], r2[:, w])
                    t = wk.tile([128, 1152], F16, name="t")
                    eng("t").tensor_mul(t[:, w], s1[:, w], inv[:, w])
                    ovp = wk.tile([128, 1152], F16, name="ovp")
                    eng("ovp").tensor_sub(ovp[:, w], t[:, w], dist[:, w])
                    # mask before max: NaN*0=NaN then max(NaN,0)=0
                    nc.vector.tensor_mul(ovp[:, 0:128], ovp[:, 0:128], msk_s[:])
                    ov = wk.tile([128, 1152], F16, name="ov")
                    with nc.allow_low_precision("fp16 chain"):
                        nc.vector.tensor_scalar_max(ov[:, w], ovp[:, w], 0.0)
                        den = wk.tile([128, 1152], F16, name="den")
                        nc.vector.scalar_tensor_tensor(
                            out=den[:, w], in0=ov[:, w], scalar=0.1,
                            in1=ones[:, w], op0=ALU.mult, op1=ALU.add,
                            accum_out=ocols[:, 3 * i:3 * i + 1])
                        rden = wk.tile([128, 1152], F16, name="rden")
                        nc.vector.reciprocal(rden[:, w], den[:, w])
                        rd2 = wk.tile([128, 1152], F16, name="rd2")
                        nc.scalar.activation(
                            rd2[:, w], rden[:, w], AF.Relu,
                            accum_out=ocols[:, 3 * i + 1:3 * i + 2])
                    h = wk.tile([128, 1152], F16, name="h")
                    eng("h").tensor_mul(h[:, w], ov[:, w], inv[:, w])
                    vta = wk.tile([128, 1152], F16, name="vta")
                    nc.vector.scalar_tensor_tensor(
                        out=vta[:, w], in0=h[:, w], scalar=1.0, in1=rva[:, w],
                        op0=ALU.mult, op1=ALU.mult,
                        accum_out=ocols[:, 3 * i + 2:3 * i + 3])

            nc.sync.dma_start(out_d[:], ocols[:])

    nc.compile()
    _NC_CACHE[key] = nc
    return nc


def make_in_maps(xyz, scales, rotations, velocities):
    l32, r32, lr, rr = _prep(xyz, scales, rotations, velocities)
    mask = (np.arange(128)[None, :] > np.arange(128)[:, None])
    mask = mask.astype(np.float16)
    in_maps = []
    for c in range(NC):
        b, half = c // 2, c % 2
        rts = ROWSETS[half]
        lcols = np.concatenate([np.arange(128 * rt, 128 * (rt + 1))
                                for rt in rts])
        r32c = np.empty((5, RHS_COLS), np.float32)
        rrc = np.zeros((96, RHS_COLS), np.float32)
        for i, rt in enumerate(rts):
            W = 1152 if rt < 8 else 1024
            cols = (128 * rt + np.arange(W)) % N
            o = STRIP_OFF[i]
            r32c[:, o:o + W] = r32[b][:, cols]
            rrc[:, o:o + W] = rr[b][:, cols]
        in_maps.append({
            "l32": np.ascontiguousarray(l32[b][:, lcols]).astype(np.float32),
            "r32": r32c,
            "lr": np.ascontiguousarray(lr[b][:, lcols]).astype(np.float32),
            "rr": rrc,
            "msk": mask,
        })
    return in_maps


def finish(results):
    total = 0.0
    C_core = 4 * 128 * 1152 + 4 * 128 * 1024
    for c in range(NC):
        o = results[c]["out"].astype(np.float64)
        sden = o[:, 0::3].sum()
        srden = o[:, 1::3].sum()
        svta = o[:, 2::3].sum()
        total += 100.0 * (sden + srden - 2.0 * C_core) + svta
    return np.float32(2.0 * total / (B * N * N))


_RUNNER = {}


def _get_runner(reps=1):
    """Cached shard_map-jitted executor (mirrors bass2jax.run_bass_via_pjrt
    multi-core path) so repeated calls skip re-compilation."""
    if reps in _RUNNER:
        return _RUNNER[reps]
    import jax
    from jax.sharding import Mesh, PartitionSpec
    from jax.experimental.shard_map import shard_map
    from concourse import bass2jax

    nc = _build(reps)
    bass2jax.install_neuronx_cc_hook()

    part_name = nc.partition_id_tensor.name if nc.partition_id_tensor else None
    in_names, out_names, out_avals, zero_outs = [], [], [], []
    for alloc in nc.m.functions[0].allocations:
        if not isinstance(alloc, mybir.MemoryLocationSet):
            continue
        name = alloc.memorylocations[0].name
        if alloc.kind == "ExternalInput":
            if name != part_name:
                in_names.append(name)
        elif alloc.kind == "ExternalOutput":
            out_names.append(name)
            shape = tuple(alloc.tensor_shape)
            dtype = mybir.dt.np(alloc.dtype)
            out_avals.append(jax.core.ShapedArray(shape, dtype))
            zero_outs.append(np.zeros(shape, dtype))
    n_params = len(in_names)
    all_names = in_names + out_names
    if part_name is not None:
        all_names = all_names + [part_name]

    def _body(*args):
        operands = list(args)
        if part_name is not None:
            operands.append(bass2jax.partition_id_tensor())
        outs = bass2jax._bass_exec_p.bind(
            *operands,
            out_avals=tuple(out_avals),
            in_names=tuple(all_names),
            out_names=tuple(out_names),
            lowering_input_output_aliases=(),
            sim_require_finite=True,
            sim_require_nnan=True,
            nc=nc,
        )
        return tuple(outs)

    devices = jax.devices()[:NC]
    mesh = Mesh(np.asarray(devices), ("core",))
    n_outs = len(out_names)
    fn = jax.jit(
        shard_map(
            _body, mesh=mesh,
            in_specs=(PartitionSpec("core"),) * (n_params + n_outs),
            out_specs=(PartitionSpec("core"),) * n_outs,
            check_rep=False,
        ),
        donate_argnums=tuple(range(n_params, n_params + n_outs)),
        keep_unused=True,
    )

    def run(in_maps):
        concat_in = [
            np.concatenate([in_maps[c][nm] for c in range(NC)], axis=0)
            for nm in in_names
        ]
        concat_zeros = [
            np.zeros((NC * z.shape[0], *z.shape[1:]), z.dtype) for z in zero_outs
        ]
        out_arrs = fn(*concat_in, *concat_zeros)
        return [
            {nm: np.asarray(out_arrs[i]).reshape(NC, *out_avals[i].shape)[c]
             for i, nm in enumerate(out_names)}
            for c in range(NC)
        ]

    _RUNNER[reps] = run
    return run


def kernel(xyz, scales, rotations, velocities):
    run = _get_runner()
    in_maps = make_in_maps(xyz, scales, rotations, velocities)
    return finish(run(in_maps))


if __name__ == "__main__":
    rng = np.random.default_rng(0)
    ins = {
        "xyz": rng.standard_normal((B, N, 3)).astype(np.float32),
        "scales": rng.random((B, N, 3)).astype(np.float32),
        "rotations": rng.standard_normal((B, N, 4)).astype(np.float32),
        "velocities": rng.standard_normal((B, N, 3)).astype(np.float32),
    }
    print(kernel(**ins))
